# revision 13
# baseline (speedup 1.0000x reference)
"""Trainium2 Bass kernel for MeanAggregator GNN message passing.

Computation (see reference):
  h = tanh(BN_trainmode(features @ W.T + b)) ; out = row-mean over sampled
  neighbor set (deduped membership mask) of h rows.

Strategy (8 cores, SPMD), rev2:
  - Shard feature-table rows (U=50000 -> 8 x 6272) across cores; fp16
    datapath on PE (fp32 runs at 1/4 rate), fp32 PSUM accumulation.
  - Phase A: mm^T = W @ x^T -> [E=128, U_local] (PE), per-channel
    sum/sumsq stats, AND immediately PE-transpose raw mm rows -> DRAM
    (no stats dependency), so gather descriptor generation can start
    ~35us in and hide the collective bootstrap + AllReduce latency.
  - Tiny AllReduce of [128,2] stats -> scale/shift; broadcast to
    [128,128] tiles via PE outer product.
  - Aggregation: deduped (row,col) entries sharded by col owner, sorted
    by row, padded per 128-row output window to groups of 128 entries.
    dma_gather pulls RAW mm rows (fp16, 256B each); per group:
    normalize+tanh applied to the gathered tile (2 DVE ops + ACT), then
    a one-hot scatter matmul (S[j,r] = w_j * (row_j == r)) accumulates
    out[r,e] per window in PSUM.
  - ReduceScatter(add) of the 8 partial [4096,128] outputs; host
    concatenates the per-core [512,128] slices.
"""

import sys

for _p in ("/opt/trn_rl_repo", "/root/.axon_site/_ro/trn_rl_repo"):
    if _p not in sys.path:
        sys.path.append(_p)

import numpy as np

import concourse.bass as bass
import concourse.bacc as bacc
import concourse.tile as tile
import concourse.mybir as mybir
from concourse.bass_utils import run_bass_kernel_spmd

F32 = mybir.dt.float32
F16 = mybir.dt.float16
I16 = mybir.dt.int16
I32 = mybir.dt.int32
AF = mybir.ActivationFunctionType
OP = mybir.AluOpType

N_CORES = 8
U, F, E, B = 50000, 256, 128, 4096
UL = 6272            # per-core feature rows (49 * 128); 8*6272 = 50176 >= U
WIN = 128            # output rows per scatter window
NWIN = B // WIN      # 32
BN_EPS = 1e-5
OUT_PART = B // N_CORES  # 512 rows of output per core after ReduceScatter
MAX_G_PER_CALL = 6   # dma_gather <= 768 idxs/call (descriptor carveout)

U_CHUNKS = [(i * 512, 512) for i in range(UL // 512)]
if UL % 512:
    U_CHUNKS.append((UL - UL % 512, UL % 512))

_CACHE = {}
LAST_RESULTS = None
TRACE = False


def _build(gw):
    key = tuple(gw)
    if key in _CACHE:
        return _CACHE[key]

    gbase = np.concatenate([[0], np.cumsum(gw)]).astype(int)
    NG = int(gbase[-1])

    nc = bacc.Bacc("TRN2", target_bir_lowering=False, debug=False,
                   enable_asserts=False, num_devices=N_CORES)

    # ---- I/O ----
    xT = nc.dram_tensor("xT", [F, UL], F16, kind="ExternalInput")
    Wt = nc.dram_tensor("Wt", [F, E], F16, kind="ExternalInput")
    gb = nc.dram_tensor("gb", [E, 2], F32, kind="ExternalInput")
    gidx = nc.dram_tensor("gidx", [128, NG * 8], I16, kind="ExternalInput")
    rowp = nc.dram_tensor("rowp", [128, NG], F32, kind="ExternalInput")
    wvec = nc.dram_tensor("wvec", [128, NG], F32, kind="ExternalInput")
    out_part = nc.dram_tensor("out_part", [OUT_PART, E], F32,
                              kind="ExternalOutput")

    # ---- internal DRAM ----
    hdram = nc.dram_tensor("hdram", [UL, E], F16)   # raw mm rows (fp16)
    stats_in = nc.dram_tensor("stats_in", [E, 2], F32)
    stats_out = nc.dram_tensor("stats_out", [E, 2], F32, addr_space="Shared")
    rs_in = nc.dram_tensor("rs_in", [B, E], F32)
    rs_out = nc.dram_tensor("rs_out", [OUT_PART, E], F32)

    RG = [list(range(N_CORES))]

    with tile.TileContext(nc) as tc:
        with (
            tc.tile_pool(name="const", bufs=1) as cpool,
            tc.tile_pool(name="rot", bufs=3) as rot,
        ):
            # ---- constants ----
            wt0 = cpool.tile([128, E], F16, tag="wt0")
            wt1 = cpool.tile([128, E], F16, tag="wt1")
            nc.sync.dma_start(wt0[:], Wt[0:128, :])
            nc.sync.dma_start(wt1[:], Wt[128:256, :])
            gbt = cpool.tile([E, 2], F32, tag="gbt")
            nc.sync.dma_start(gbt[:], gb[:])
            idxt = cpool.tile([128, NG * 8], I16, tag="idxt")
            nc.sync.dma_start(idxt[:], gidx[:])
            rowpt = cpool.tile([128, NG], F32, tag="rowpt")
            nc.sync.dma_start(rowpt[:], rowp[:])
            wvt = cpool.tile([128, NG], F32, tag="wvt")
            nc.sync.dma_start(wvt[:], wvec[:])

            iota_i = cpool.tile([128, 128], I32, tag="iota_i")
            nc.gpsimd.iota(iota_i[:], pattern=[[1, 128]], base=0,
                           channel_multiplier=0)
            iota_f = cpool.tile([128, 128], F32, tag="iota_f")
            nc.vector.tensor_copy(iota_f[:], iota_i[:])
            iden_i = cpool.tile([128, 128], I32, tag="iden_i")
            nc.gpsimd.iota(iden_i[:], pattern=[[1, 128]], base=0,
                           channel_multiplier=-1)
            iden_f0 = cpool.tile([128, 128], F32, tag="iden_f0")
            nc.vector.tensor_copy(iden_f0[:], iden_i[:])
            ident = cpool.tile([128, 128], F32, tag="ident")
            nc.vector.tensor_scalar(ident[:], iden_f0[:], 0.0, None,
                                    op0=OP.is_equal)
            ident_h = cpool.tile([128, 128], F16, tag="ident_h")
            nc.vector.tensor_copy(ident_h[:], ident[:])
            zcol = cpool.tile([128, 1], F32, tag="zcol")
            nc.vector.memset(zcol[:], 0.0)
            epscol = cpool.tile([128, 1], F32, tag="epscol")
            nc.vector.memset(epscol[:], BN_EPS)
            ones_row = cpool.tile([1, 128], F32, tag="ones_row")
            nc.vector.memset(ones_row[:], 1.0)

            n_ch = len(U_CHUNKS)
            musum_cols = cpool.tile([128, n_ch], F32, tag="musum")
            ssq_cols = cpool.tile([128, n_ch], F32, tag="ssq")

            # ---- phase A: matmul, stats, transpose raw mm -> hdram ----
            with tc.tile_pool(name="psA", bufs=1, space="PSUM") as psA:
                for ci, (u0, un) in enumerate(U_CHUNKS):
                    x0 = rot.tile([128, un], F16, tag="x0")
                    x1 = rot.tile([128, un], F16, tag="x1")
                    nc.sync.dma_start(x0[:], xT[0:128, u0:u0 + un])
                    nc.sync.dma_start(x1[:], xT[128:256, u0:u0 + un])
                    ps = psA.tile([128, un], F32, tag=f"ps{ci % 2}")
                    nc.tensor.matmul(ps[:], wt0[:], x0[:],
                                     start=True, stop=False)
                    nc.tensor.matmul(ps[:], wt1[:], x1[:],
                                     start=False, stop=True)
                    mm = rot.tile([128, un], F16, tag="mm")
                    nc.vector.tensor_copy(mm[:], ps[:])
                    nc.vector.tensor_reduce(
                        musum_cols[:, ci:ci + 1], mm[:],
                        axis=mybir.AxisListType.X, op=OP.add)
                    sq = rot.tile([128, un], F16, tag="sq")
                    nc.scalar.activation(sq[:], mm[:], AF.Square,
                                         bias=zcol[:, 0:1],
                                         accum_out=ssq_cols[:, ci:ci + 1])
                    for s in range(0, un, 128):
                        tp = psA.tile([128, 128], F16,
                                      tag=f"tp{(s // 128) % 2}")
                        nc.tensor.transpose(tp[:], mm[:, s:s + 128],
                                            ident_h[:])
                        hsb = rot.tile([128, 128], F16, tag="hsb")
                        nc.vector.tensor_copy(hsb[:], tp[:])
                        nc.sync.dma_start(hdram[u0 + s:u0 + s + 128, :],
                                          hsb[:])

            # ---- stats AllReduce ----
            stats_sb = cpool.tile([E, 2], F32, tag="stats_sb")
            nc.vector.tensor_reduce(stats_sb[:, 0:1], musum_cols[:],
                                    axis=mybir.AxisListType.X, op=OP.add)
            nc.vector.tensor_reduce(stats_sb[:, 1:2], ssq_cols[:],
                                    axis=mybir.AxisListType.X, op=OP.add)
            nc.sync.dma_start(stats_in[:], stats_sb[:])
            nc.gpsimd.collective_compute(
                "AllReduce", OP.add, replica_groups=RG,
                ins=[stats_in.ap()], outs=[stats_out.ap()])
            stats_g = cpool.tile([E, 2], F32, tag="stats_g")
            nc.sync.dma_start(stats_g[:], stats_out[:])

            # scale/shift per channel (as [E,1] columns)
            mu = cpool.tile([E, 1], F32, tag="mu")
            nc.vector.tensor_scalar_mul(mu[:], stats_g[:, 0:1], 1.0 / U)
            ex2 = cpool.tile([E, 1], F32, tag="ex2")
            nc.vector.tensor_scalar_mul(ex2[:], stats_g[:, 1:2], 1.0 / U)
            musq = cpool.tile([E, 1], F32, tag="musq")
            nc.vector.tensor_tensor(musq[:], mu[:], mu[:], op=OP.mult)
            var = cpool.tile([E, 1], F32, tag="var")
            nc.vector.tensor_tensor(var[:], ex2[:], musq[:], op=OP.subtract)
            sd = cpool.tile([E, 1], F32, tag="sd")
            nc.scalar.activation(sd[:], var[:], AF.Sqrt,
                                 bias=epscol[:, 0:1])
            rinv = cpool.tile([E, 1], F32, tag="rinv")
            nc.vector.reciprocal(rinv[:], sd[:])
            ss_col = cpool.tile([E, 2], F32, tag="ss_col")
            nc.vector.tensor_tensor(ss_col[:, 0:1], rinv[:], gbt[:, 0:1],
                                    op=OP.mult)  # scale
            msc = cpool.tile([E, 1], F32, tag="msc")
            nc.vector.tensor_tensor(msc[:], mu[:], ss_col[:, 0:1],
                                    op=OP.mult)
            nc.vector.tensor_tensor(ss_col[:, 1:2], gbt[:, 1:2], msc[:],
                                    op=OP.subtract)  # shift

            # broadcast scale/shift to [128,128] via PE:
            # ssrow = ss_col^T (transpose), bc = ones_col x ssrow (outer)
            with tc.tile_pool(name="psS", bufs=1, space="PSUM") as psS:
                scr_ps = psS.tile([1, 128], F32, tag="scr")
                nc.tensor.transpose(scr_ps[:], ss_col[:, 0:1], ident[:])
                sc_row = cpool.tile([1, 128], F32, tag="sc_row")
                nc.vector.tensor_copy(sc_row[:], scr_ps[:])
                shr_ps = psS.tile([1, 128], F32, tag="shr")
                nc.tensor.transpose(shr_ps[:], ss_col[:, 1:2], ident[:])
                sh_row = cpool.tile([1, 128], F32, tag="sh_row")
                nc.vector.tensor_copy(sh_row[:], shr_ps[:])
                sc_ps = psS.tile([128, 128], F32, tag="scps")
                nc.tensor.matmul(sc_ps[:], ones_row[:], sc_row[:],
                                 start=True, stop=True)
                scale_bc = cpool.tile([128, 128], F32, tag="scale_bc")
                nc.vector.tensor_copy(scale_bc[:], sc_ps[:])
                sh_ps = psS.tile([128, 128], F32, tag="shps")
                nc.tensor.matmul(sh_ps[:], ones_row[:], sh_row[:],
                                 start=True, stop=True)
                shift_bc = cpool.tile([128, 128], F32, tag="shift_bc")
                nc.vector.tensor_copy(shift_bc[:], sh_ps[:])

            # ---- phase C: gather raw rows + normalize + scatter-matmul ----
            win_chunks = []
            cur, cur_g = [], 0
            for w in range(NWIN):
                if cur and cur_g + int(gw[w]) > MAX_G_PER_CALL:
                    win_chunks.append(cur)
                    cur, cur_g = [], 0
                cur.append(w)
                cur_g += int(gw[w])
            if cur:
                win_chunks.append(cur)

            with tc.tile_pool(name="psC", bufs=1, space="PSUM") as psC:
                for wc in win_chunks:
                    w_lo, w_hi = int(wc[0]), int(wc[-1]) + 1
                    g_lo, g_hi = int(gbase[w_lo]), int(gbase[w_hi])
                    ngc = g_hi - g_lo
                    if ngc == 0:
                        continue
                    gt = rot.tile([128, ngc, 128], F16, tag="gt")
                    nc.gpsimd.dma_gather(
                        out_ap=gt[:, :, :], in_ap=hdram.ap(),
                        idxs_ap=idxt[:, g_lo * 8:g_hi * 8],
                        num_idxs=ngc * 128, num_idxs_reg=ngc * 128,
                        elem_size=E)
                    for w in range(w_lo, w_hi):
                        wp = psC.tile([128, E], F32, tag=f"wp{w % 8}")
                        ng_w = int(gw[w])
                        for gi in range(ng_w):
                            g = int(gbase[w]) + gi
                            # normalize + tanh on the gathered raw rows
                            t1 = rot.tile([128, 128], F16, tag="t1")
                            nc.vector.tensor_tensor(
                                t1[:], gt[:, g - g_lo, :], scale_bc[:],
                                op=OP.mult)
                            t2 = rot.tile([128, 128], F16, tag="t2")
                            nc.vector.tensor_tensor(
                                t2[:], t1[:], shift_bc[:], op=OP.add)
                            gn = rot.tile([128, 128], F16, tag="gn")
                            nc.scalar.activation(gn[:], t2[:], AF.Tanh,
                                                 bias=zcol[:, 0:1])
                            s_t = rot.tile([128, 128], F16, tag="s_t")
                            nc.vector.tensor_scalar(
                                s_t[:], iota_f[:], rowpt[:, g:g + 1],
                                wvt[:, g:g + 1],
                                op0=OP.is_equal, op1=OP.mult)
                            nc.tensor.matmul(wp[:], s_t[:], gn[:],
                                             start=(gi == 0),
                                             stop=(gi == ng_w - 1))
                        osb = rot.tile([128, E], F32, tag="osb")
                        nc.vector.tensor_copy(osb[:], wp[:])
                        nc.sync.dma_start(rs_in[w * WIN:(w + 1) * WIN, :],
                                          osb[:])

            # ---- phase D: ReduceScatter + output ----
            nc.gpsimd.collective_compute(
                "ReduceScatter", OP.add, replica_groups=RG,
                ins=[rs_in.ap()], outs=[rs_out.ap()])
            nc.sync.dma_start(out_part.ap(), rs_out.ap())

    nc.compile()
    _CACHE[key] = nc
    return nc


def _prep_inputs(features, W, b, gamma, beta, row_idx, col_idx):
    """Host-side sharding / index preprocessing. Returns (gw, in_maps)."""
    features = np.asarray(features, dtype=np.float32)
    W = np.asarray(W, dtype=np.float32)
    gamma = np.asarray(gamma, dtype=np.float32)
    beta = np.asarray(beta, dtype=np.float32)
    row = np.asarray(row_idx).astype(np.int64)
    col = np.asarray(col_idx).astype(np.int64)

    # dedup (row, col) pairs: mask "set" semantics
    key = row * np.int64(U) + col
    order = np.argsort(key, kind="stable")
    sk = key[order]
    keep_s = np.ones(len(sk), dtype=bool)
    keep_s[1:] = sk[1:] != sk[:-1]
    keep = np.zeros(len(key), dtype=bool)
    keep[order] = keep_s
    urow = row[keep]
    ucol = col[keep]
    cnt = np.bincount(urow, minlength=B)
    wgt = (1.0 / np.maximum(cnt, 1)[urow]).astype(np.float32)

    Wt_full = np.ascontiguousarray(W.T).astype(np.float16)
    gb_full = np.ascontiguousarray(np.stack([gamma, beta], axis=1))

    percore = []
    cw_all = np.zeros((N_CORES, NWIN), dtype=np.int64)
    for k in range(N_CORES):
        sel = (ucol >= k * UL) & (ucol < (k + 1) * UL)
        rk = urow[sel]
        ck = (ucol[sel] - k * UL).astype(np.int16)
        wk = wgt[sel]
        o = np.argsort(rk, kind="stable")
        rk, ck, wk = rk[o], ck[o], wk[o]
        cw = np.bincount(rk // WIN, minlength=NWIN)
        cw_all[k] = cw
        percore.append((rk, ck, wk, cw))

    gw = np.maximum(1, -(-cw_all.max(axis=0) // 128))  # ceil, >= 1
    gbase = np.concatenate([[0], np.cumsum(gw)]).astype(int)
    NG = int(gbase[-1])

    in_maps = []
    for k in range(N_CORES):
        rk, ck, wk, cw = percore[k]
        idx_flat = np.zeros(NG * 128, dtype=np.int16)
        rowp_a = np.full((128, NG), -1.0, dtype=np.float32)
        wv_a = np.zeros((128, NG), dtype=np.float32)
        cstart = np.concatenate([[0], np.cumsum(cw)]).astype(int)
        for w in range(NWIN):
            n = int(cw[w])
            if n == 0:
                continue
            e0 = cstart[w]
            pos = gbase[w] * 128 + np.arange(n)
            idx_flat[pos] = ck[e0:e0 + n]
            rowp_a[pos % 128, pos // 128] = (rk[e0:e0 + n] - WIN * w)
            wv_a[pos % 128, pos // 128] = wk[e0:e0 + n]
        # idxs live wrapped in 16 partitions, replicated across the 8 Q7 cores
        idx_packed = np.tile(idx_flat.reshape(NG * 8, 16).T, (8, 1))

        xpart = np.zeros((UL, F), dtype=np.float32)
        lo, hi = k * UL, min((k + 1) * UL, U)
        if hi > lo:
            xpart[:hi - lo] = features[lo:hi]
        xT_k = np.ascontiguousarray(xpart.T).astype(np.float16)

        in_maps.append({
            "xT": xT_k,
            "Wt": Wt_full,
            "gb": gb_full,
            "gidx": idx_packed,
            "rowp": rowp_a,
            "wvec": wv_a,
        })
    return gw, in_maps


def kernel(features, W, b, gamma, beta, row_idx, col_idx, B=4096):
    global LAST_RESULTS
    gw, in_maps = _prep_inputs(features, W, b, gamma, beta, row_idx, col_idx)
    nc = _build(tuple(int(g) for g in gw))
    res = run_bass_kernel_spmd(nc, in_maps, list(range(N_CORES)), trace=TRACE)
    LAST_RESULTS = res
    out = np.concatenate([res.results[c]["out_part"] for c in range(N_CORES)],
                         axis=0)
    return out


# revision 15
# speedup vs baseline: 1.2038x; 1.2038x over previous
"""Trainium2 Bass kernel for MeanAggregator GNN message passing.

Computation (see reference):
  h = tanh(BN_trainmode(features @ W.T + b)) ; out = row-mean over sampled
  neighbor set (deduped membership mask) of h rows.

Strategy (8 cores, SPMD), rev3:
  - Shard feature-table rows (U=50000 -> 8 x 6272) across cores; fp16
    datapath (PE fp32 runs at 1/4 rate; mixed-dtype DVE ops are ~10x
    slower than uniform fp16), fp32 PSUM accumulation and BN stats.
  - Phase A: mm^T = W @ x^T (PE), per-channel sum/sumsq stats, and
    immediate PE-transpose of raw mm rows -> DRAM (no stats dep).
  - Tiny AllReduce of [128,2] stats; its latency (and the collective
    bootstrap barrier) hides under the gather descriptor generation,
    which is the serial bottleneck (~7us of GpSimd Q7 time per 768
    indices).
  - Aggregation: deduped (row,col) entries sharded by col owner, sorted
    by row, padded per 128-row output window to groups of 128 entries.
    dma_gather pulls raw fp16 mm rows; per group: normalize+tanh on the
    gathered tile (fp16 DVE+ACT), then a one-hot scatter matmul
    (S[j,r] = w_j * (row_j == r)) accumulates out[r,e] in PSUM.
  - ReduceScatter(add) of the 8 partial [4096,128] outputs; host
    concatenates the per-core [512,128] slices.
"""

import sys

for _p in ("/opt/trn_rl_repo", "/root/.axon_site/_ro/trn_rl_repo"):
    if _p not in sys.path:
        sys.path.append(_p)

import numpy as np

import concourse.bass as bass
import concourse.bacc as bacc
import concourse.tile as tile
import concourse.mybir as mybir
from concourse.bass_utils import run_bass_kernel_spmd

F32 = mybir.dt.float32
F16 = mybir.dt.float16
I16 = mybir.dt.int16
I32 = mybir.dt.int32
AF = mybir.ActivationFunctionType
OP = mybir.AluOpType

N_CORES = 8
U, F, E, B = 50000, 256, 128, 4096
UL = 6272            # per-core feature rows (49 * 128); 8*6272 = 50176 >= U
WIN = 128            # output rows per scatter window
NWIN = B // WIN      # 32
BN_EPS = 1e-5
OUT_PART = B // N_CORES
MAX_G_PER_CALL = 6   # dma_gather <= 768 idxs/call (descriptor carveout)

U_CHUNKS = [(i * 512, 512) for i in range(UL // 512)]
if UL % 512:
    U_CHUNKS.append((UL - UL % 512, UL % 512))

_CACHE = {}
LAST_RESULTS = None
TRACE = False


def _build(gw):
    key = tuple(gw)
    if key in _CACHE:
        return _CACHE[key]

    gbase = np.concatenate([[0], np.cumsum(gw)]).astype(int)
    NG = int(gbase[-1])

    nc = bacc.Bacc("TRN2", target_bir_lowering=False, debug=False,
                   enable_asserts=False, num_devices=N_CORES)

    # ---- I/O ----
    xT = nc.dram_tensor("xT", [F, UL], F16, kind="ExternalInput")
    Wt = nc.dram_tensor("Wt", [F, E], F16, kind="ExternalInput")
    gb = nc.dram_tensor("gb", [E, 2], F32, kind="ExternalInput")
    gidx = nc.dram_tensor("gidx", [128, NG * 8], I16, kind="ExternalInput")
    smat = nc.dram_tensor("smat", [128, NG * 128], F16, kind="ExternalInput")
    out_part = nc.dram_tensor("out_part", [OUT_PART, E], F32,
                              kind="ExternalOutput")

    # ---- internal DRAM ----
    hdram = nc.dram_tensor("hdram", [UL, E], F16)   # raw mm rows
    stats_in = nc.dram_tensor("stats_in", [E, 2], F32)
    stats_out = nc.dram_tensor("stats_out", [E, 2], F32, addr_space="Shared")
    rs_in = nc.dram_tensor("rs_in", [B, E], F32)
    rs_out = nc.dram_tensor("rs_out", [OUT_PART, E], F32)

    RG = [list(range(N_CORES))]

    win_chunks = []
    cur, cur_g = [], 0
    for w in range(NWIN):
        if cur and cur_g + int(gw[w]) > MAX_G_PER_CALL:
            win_chunks.append(cur)
            cur, cur_g = [], 0
        cur.append(w)
        cur_g += int(gw[w])
    if cur:
        win_chunks.append(cur)

    with tile.TileContext(nc) as tc:
        with (
            tc.tile_pool(name="const", bufs=1) as cpool,
            tc.tile_pool(name="rot", bufs=3) as rot,
        ):
            # ---- constants ----
            wt0 = cpool.tile([128, E], F16, tag="wt0")
            wt1 = cpool.tile([128, E], F16, tag="wt1")
            nc.sync.dma_start(wt0[:], Wt[0:128, :])
            nc.sync.dma_start(wt1[:], Wt[128:256, :])
            gbt = cpool.tile([E, 2], F32, tag="gbt")
            nc.sync.dma_start(gbt[:], gb[:])
            idxt = cpool.tile([128, NG * 8], I16, tag="idxt")
            nc.sync.dma_start(idxt[:], gidx[:])
            smt = cpool.tile([128, NG * 128], F16, tag="smt")
            nc.sync.dma_start(smt[:], smat[:])

            iden_i = cpool.tile([128, 128], I32, tag="iden_i")
            nc.gpsimd.iota(iden_i[:], pattern=[[1, 128]], base=0,
                           channel_multiplier=-1)
            iden_f0 = cpool.tile([128, 128], F32, tag="iden_f0")
            nc.vector.tensor_copy(iden_f0[:], iden_i[:])
            ident = cpool.tile([128, 128], F32, tag="ident")
            nc.vector.tensor_scalar(ident[:], iden_f0[:], 0.0, None,
                                    op0=OP.is_equal)
            ident_h = cpool.tile([128, 128], F16, tag="ident_h")
            nc.vector.tensor_copy(ident_h[:], ident[:])
            zcol = cpool.tile([128, 1], F32, tag="zcol")
            nc.vector.memset(zcol[:], 0.0)
            epscol = cpool.tile([128, 1], F32, tag="epscol")
            nc.vector.memset(epscol[:], BN_EPS)
            ones_row = cpool.tile([1, 128], F32, tag="ones_row")
            nc.vector.memset(ones_row[:], 1.0)

            n_ch = len(U_CHUNKS)
            musum_cols = cpool.tile([128, n_ch], F32, tag="musum")
            ssq_cols = cpool.tile([128, n_ch], F32, tag="ssq")

            # whole-half xT loads (two big DMAs; matmuls slice them)
            xt0 = cpool.tile([128, UL], F16, tag="xt0")
            xt1 = cpool.tile([128, UL], F16, tag="xt1")
            nc.sync.dma_start(xt0[:], xT[0:128, :])
            nc.sync.dma_start(xt1[:], xT[128:256, :])

            # ---- phase A: matmul, stats, transpose raw mm -> hdram ----
            with tc.tile_pool(name="psA", bufs=1, space="PSUM") as psA:
                for ci, (u0, un) in enumerate(U_CHUNKS):
                    ps = psA.tile([128, un], F32, tag=f"ps{ci % 2}")
                    nc.tensor.matmul(ps[:], wt0[:], xt0[:, u0:u0 + un],
                                     start=True, stop=False)
                    nc.tensor.matmul(ps[:], wt1[:], xt1[:, u0:u0 + un],
                                     start=False, stop=True)
                    mm = rot.tile([128, un], F16, tag="mm")
                    nc.vector.tensor_copy(mm[:], ps[:])
                    nc.vector.tensor_reduce(
                        musum_cols[:, ci:ci + 1], mm[:],
                        axis=mybir.AxisListType.X, op=OP.add)
                    sq = rot.tile([128, un], F16, tag="sq")
                    nc.scalar.activation(sq[:], mm[:], AF.Square,
                                         bias=zcol[:, 0:1],
                                         accum_out=ssq_cols[:, ci:ci + 1])
                    nb = un // 128
                    hsb = rot.tile([128, nb, 128], F16, tag="hsb")
                    for b in range(nb):
                        tp = psA.tile([128, 128], F16, tag=f"tp{b % 2}")
                        nc.tensor.transpose(
                            tp[:], mm[:, b * 128:(b + 1) * 128], ident_h[:])
                        nc.vector.tensor_copy(hsb[:, b, :], tp[:])
                    nc.sync.dma_start(
                        hdram[u0:u0 + un, :].rearrange(
                            "(b p) e -> p b e", p=128), hsb[:])

            # ---- stats AllReduce (trigger before gathers on gpsimd) ----
            stats_sb = cpool.tile([E, 2], F32, tag="stats_sb")
            nc.vector.tensor_reduce(stats_sb[:, 0:1], musum_cols[:],
                                    axis=mybir.AxisListType.X, op=OP.add)
            nc.vector.tensor_reduce(stats_sb[:, 1:2], ssq_cols[:],
                                    axis=mybir.AxisListType.X, op=OP.add)
            nc.sync.dma_start(stats_in[:], stats_sb[:])
            nc.gpsimd.collective_compute(
                "AllReduce", OP.add, replica_groups=RG,
                ins=[stats_in.ap()], outs=[stats_out.ap()])

            # ---- all gathers back-to-back (the serial Q7 bottleneck) ----
            gts = []
            for ci_, wc in enumerate(win_chunks):
                w_lo, w_hi = int(wc[0]), int(wc[-1]) + 1
                g_lo, g_hi = int(gbase[w_lo]), int(gbase[w_hi])
                ngc = g_hi - g_lo
                gt = cpool.tile([128, ngc, 128], F16, tag=f"gt{ci_}")
                nc.gpsimd.dma_gather(
                    out_ap=gt[:, :, :], in_ap=hdram.ap(),
                    idxs_ap=idxt[:, g_lo * 8:g_hi * 8],
                    num_idxs=ngc * 128, num_idxs_reg=ngc * 128,
                    elem_size=E)
                gts.append(gt)

            # ---- scale/shift from allreduced stats ----
            stats_g = cpool.tile([E, 2], F32, tag="stats_g")
            nc.sync.dma_start(stats_g[:], stats_out[:])
            mu = cpool.tile([E, 1], F32, tag="mu")
            nc.vector.tensor_scalar_mul(mu[:], stats_g[:, 0:1], 1.0 / U)
            ex2 = cpool.tile([E, 1], F32, tag="ex2")
            nc.vector.tensor_scalar_mul(ex2[:], stats_g[:, 1:2], 1.0 / U)
            musq = cpool.tile([E, 1], F32, tag="musq")
            nc.vector.tensor_tensor(musq[:], mu[:], mu[:], op=OP.mult)
            var = cpool.tile([E, 1], F32, tag="var")
            nc.vector.tensor_tensor(var[:], ex2[:], musq[:], op=OP.subtract)
            sd = cpool.tile([E, 1], F32, tag="sd")
            nc.scalar.activation(sd[:], var[:], AF.Sqrt, bias=epscol[:, 0:1])
            rinv = cpool.tile([E, 1], F32, tag="rinv")
            nc.vector.reciprocal(rinv[:], sd[:])
            ss_col = cpool.tile([E, 2], F32, tag="ss_col")
            nc.vector.tensor_tensor(ss_col[:, 0:1], rinv[:], gbt[:, 0:1],
                                    op=OP.mult)
            msc = cpool.tile([E, 1], F32, tag="msc")
            nc.vector.tensor_tensor(msc[:], mu[:], ss_col[:, 0:1],
                                    op=OP.mult)
            nc.vector.tensor_tensor(ss_col[:, 1:2], gbt[:, 1:2], msc[:],
                                    op=OP.subtract)

            # broadcast scale/shift to fp16 [128,128] tiles via PE
            with tc.tile_pool(name="psS", bufs=1, space="PSUM") as psS:
                scr_ps = psS.tile([1, 128], F32, tag="scr")
                nc.tensor.transpose(scr_ps[:], ss_col[:, 0:1], ident[:])
                sc_row = cpool.tile([1, 128], F32, tag="sc_row")
                nc.vector.tensor_copy(sc_row[:], scr_ps[:])
                shr_ps = psS.tile([1, 128], F32, tag="shr")
                nc.tensor.transpose(shr_ps[:], ss_col[:, 1:2], ident[:])
                sh_row = cpool.tile([1, 128], F32, tag="sh_row")
                nc.vector.tensor_copy(sh_row[:], shr_ps[:])
                sc_ps = psS.tile([128, 128], F32, tag="scps")
                nc.tensor.matmul(sc_ps[:], ones_row[:], sc_row[:],
                                 start=True, stop=True)
                scale_bc = cpool.tile([128, 128], F16, tag="scale_bc")
                nc.vector.tensor_copy(scale_bc[:], sc_ps[:])
                sh_ps = psS.tile([128, 128], F32, tag="shps")
                nc.tensor.matmul(sh_ps[:], ones_row[:], sh_row[:],
                                 start=True, stop=True)
                shift_bc = cpool.tile([128, 128], F16, tag="shift_bc")
                nc.vector.tensor_copy(shift_bc[:], sh_ps[:])

            # ---- per-window: normalize gathered rows + scatter matmul ----
            with tc.tile_pool(name="psC", bufs=1, space="PSUM") as psC:
                for ci_, wc in enumerate(win_chunks):
                    w_lo, w_hi = int(wc[0]), int(wc[-1]) + 1
                    g_lo = int(gbase[w_lo])
                    gt = gts[ci_]
                    for w in range(w_lo, w_hi):
                        wp = psC.tile([128, E], F32, tag=f"wp{w % 8}")
                        ng_w = int(gw[w])
                        for gi in range(ng_w):
                            g = int(gbase[w]) + gi
                            t1 = rot.tile([128, 128], F16, tag="t1")
                            nc.vector.tensor_tensor(
                                t1[:], gt[:, g - g_lo, :], scale_bc[:],
                                op=OP.mult)
                            t2 = rot.tile([128, 128], F16, tag="t2")
                            nc.vector.tensor_tensor(
                                t2[:], t1[:], shift_bc[:], op=OP.add)
                            gn = rot.tile([128, 128], F16, tag="gn")
                            nc.scalar.activation(gn[:], t2[:], AF.Tanh,
                                                 bias=zcol[:, 0:1])
                            nc.tensor.matmul(
                                wp[:], smt[:, g * 128:(g + 1) * 128], gn[:],
                                start=(gi == 0), stop=(gi == ng_w - 1))
                        osb = rot.tile([128, E], F32, tag="osb")
                        nc.vector.tensor_copy(osb[:], wp[:])
                        nc.sync.dma_start(rs_in[w * WIN:(w + 1) * WIN, :],
                                          osb[:])

            # ---- ReduceScatter + output ----
            nc.gpsimd.collective_compute(
                "ReduceScatter", OP.add, replica_groups=RG,
                ins=[rs_in.ap()], outs=[rs_out.ap()])
            nc.sync.dma_start(out_part.ap(), rs_out.ap())

    nc.compile()
    _CACHE[key] = nc
    return nc


def _prep_inputs(features, W, b, gamma, beta, row_idx, col_idx):
    """Host-side sharding / index preprocessing. Returns (gw, in_maps)."""
    features = np.asarray(features, dtype=np.float32)
    W = np.asarray(W, dtype=np.float32)
    gamma = np.asarray(gamma, dtype=np.float32)
    beta = np.asarray(beta, dtype=np.float32)
    row = np.asarray(row_idx).astype(np.int64)
    col = np.asarray(col_idx).astype(np.int64)

    # dedup (row, col) pairs: mask "set" semantics
    key = row * np.int64(U) + col
    order = np.argsort(key, kind="stable")
    sk = key[order]
    keep_s = np.ones(len(sk), dtype=bool)
    keep_s[1:] = sk[1:] != sk[:-1]
    keep = np.zeros(len(key), dtype=bool)
    keep[order] = keep_s
    urow = row[keep]
    ucol = col[keep]
    cnt = np.bincount(urow, minlength=B)
    wgt = (1.0 / np.maximum(cnt, 1)[urow]).astype(np.float32)

    Wt_full = np.ascontiguousarray(W.T).astype(np.float16)
    gb_full = np.ascontiguousarray(np.stack([gamma, beta], axis=1))

    percore = []
    cw_all = np.zeros((N_CORES, NWIN), dtype=np.int64)
    for k in range(N_CORES):
        sel = (ucol >= k * UL) & (ucol < (k + 1) * UL)
        rk = urow[sel]
        ck = (ucol[sel] - k * UL).astype(np.int16)
        wk = wgt[sel]
        o = np.argsort(rk, kind="stable")
        rk, ck, wk = rk[o], ck[o], wk[o]
        cw = np.bincount(rk // WIN, minlength=NWIN)
        cw_all[k] = cw
        percore.append((rk, ck, wk, cw))

    gw = np.maximum(1, -(-cw_all.max(axis=0) // 128))  # ceil, >= 1
    gbase = np.concatenate([[0], np.cumsum(gw)]).astype(int)
    NG = int(gbase[-1])

    in_maps = []
    for k in range(N_CORES):
        rk, ck, wk, cw = percore[k]
        idx_flat = np.zeros(NG * 128, dtype=np.int16)
        s_all = np.zeros((NG * 128, 128), dtype=np.float16)
        cstart = np.concatenate([[0], np.cumsum(cw)]).astype(int)
        for w in range(NWIN):
            n = int(cw[w])
            if n == 0:
                continue
            e0 = cstart[w]
            pos = gbase[w] * 128 + np.arange(n)
            idx_flat[pos] = ck[e0:e0 + n]
            s_all[pos, rk[e0:e0 + n] - WIN * w] = wk[e0:e0 + n]
        # smat[p, g*128 + r] = s_all[g*128 + p, r]
        smat_a = np.ascontiguousarray(
            s_all.reshape(NG, 128, 128).transpose(1, 0, 2).reshape(
                128, NG * 128))
        # idxs live wrapped in 16 partitions, replicated across the 8 Q7 cores
        idx_packed = np.tile(idx_flat.reshape(NG * 8, 16).T, (8, 1))

        xpart = np.zeros((UL, F), dtype=np.float32)
        lo, hi = k * UL, min((k + 1) * UL, U)
        if hi > lo:
            xpart[:hi - lo] = features[lo:hi]
        xT_k = np.ascontiguousarray(xpart.T).astype(np.float16)

        in_maps.append({
            "xT": xT_k,
            "Wt": Wt_full,
            "gb": gb_full,
            "gidx": idx_packed,
            "smat": smat_a,
        })
    return gw, in_maps


def kernel(features, W, b, gamma, beta, row_idx, col_idx, B=4096):
    global LAST_RESULTS
    gw, in_maps = _prep_inputs(features, W, b, gamma, beta, row_idx, col_idx)
    nc = _build(tuple(int(g) for g in gw))
    res = run_bass_kernel_spmd(nc, in_maps, list(range(N_CORES)), trace=TRACE)
    LAST_RESULTS = res
    out = np.concatenate([res.results[c]["out_part"] for c in range(N_CORES)],
                         axis=0)
    return out
